# revision 59
# baseline (speedup 1.0000x reference)
"""Trainium2 Bass kernel for nn_BilinearDecoder (B=131072, D=64, 32 relations).

score_b = e1_b^T R[rels_b] e2_b

Strategy
--------
Data-parallel over 8 NeuronCores (16384 samples/core). On the host, each
core's samples are counting-sorted by relation id and padded so every
relation owns a fixed-capacity column range. Relations are paired
(r, r+16): relation r lives on SBUF partitions 0..63 ("top half"),
relation r+16 on partitions 64..127 ("bottom half"), so every engine runs
at the full 128-partition width.

Per relation pair p the device computes, with a single 128x128
block-diagonal stationary weight W_p = diag(R_p, R_{p+16}):

    t[:, b]  = W_p^T e1t[:, b]          (TensorE, bf16 in / fp32 PSUM)
    v[:, b]  = t[:, b] * e2t[:, b]      (VectorE, PSUM x SBUF -> SBUF)
    score_b  = ones_half^T v[:, b]      (TensorE reduce with [128,2] ones)

Everything data-dependent (the gather by relation id) happens in host
numpy as part of sharding; the device only does dense work. The reference
value of each padded column is 0 and is discarded on the host, which also
inverts the permutation to restore the original sample order.
"""

import numpy as np

B = 131072
D = 64
NUM_REL = 32
NCORES = 8
BSH = B // NCORES  # 16384 samples per core
NPAIRS = NUM_REL // 2  # 16 relation pairs -> halves of the 128 partitions

# Module-level switches test.py may flip; the graded entry point is kernel().
TRACE = False
LAST_RESULTS = None

# Dtype configuration: stage-1 matmul operands, intermediate v (reduce
# matmul operands), and the e2 vector-multiply operand. All-bf16 halves
# HBM traffic and runs every matmul at 1 cycle/row; measured end-to-end
# scaled error vs the fp32 reference is ~3e-3 (fp32 PSUM accumulation).
STAGE1_DT = "bfloat16"
V_DT = "bfloat16"
E2_DT = "bfloat16"

_compiled_cache = {}


def _np_dt(name):
    if name == "bfloat16":
        import ml_dtypes

        return ml_dtypes.bfloat16
    return np.float32  # float32 and float32r share the numpy layout


def _build_nc(C, stage1_dt="bfloat16", v_dt="bfloat16", e2_dt="bfloat16"):
    """Build + compile the SPMD program for per-relation capacity C.

    C is a multiple of 64. Column grid per core: W = NPAIRS * C, where
    pair p owns columns [p*C, (p+1)*C) and carries relations p (top 64
    partitions) and p+16 (bottom 64 partitions).

    stage1_dt: dtype of e1 / weights feeding the block-diagonal matmul.
    v_dt: dtype of the intermediate v (= t * e2) feeding the ones-reduce
          matmul. bf16 runs the reduce at 1 cycle/row (fp32r runs at 4).
    e2_dt: dtype of the streamed e2 operand of the vector multiply.
    """
    import concourse.bass as bass  # noqa: F401  (registers engine classes)
    import concourse.mybir as mybir
    import concourse.tile as tile
    from concourse import bacc

    W = NPAIRS * C
    n_chunks = W // 512  # W = 16*C, C % 64 == 0 -> W % 1024 == 0
    mm_dt = getattr(mybir.dt, stage1_dt)
    vv_dt = getattr(mybir.dt, v_dt)
    ee_dt = getattr(mybir.dt, e2_dt)

    nc = bacc.Bacc("TRN2", debug=False, num_devices=NCORES)

    # The BIR verifier requires fp32r matmul operands' producers to emit
    # fp32r-rounded values, so PE-feeding tensors are declared in the
    # matmul dtype end-to-end (fp32r has the same 4-byte numpy layout).
    e1t_d = nc.dram_tensor("e1t", [128, W], mm_dt, kind="ExternalInput").ap()
    e2t_d = nc.dram_tensor("e2t", [128, W], ee_dt, kind="ExternalInput").ap()
    wst_d = nc.dram_tensor(
        "wst", [128, NPAIRS * 128], mm_dt, kind="ExternalInput"
    ).ap()

    # Score layout: row 0 = top-half relations, row 1 = bottom-half.
    # (Compute-engine APs must start at a partition multiple of 32, so the
    # scores stay on partitions 0-1.)
    out_d = nc.dram_tensor("out", [2, W], mybir.dt.float32, kind="ExternalOutput").ap()

    with tile.TileContext(nc) as tc:
        with (
            tc.tile_pool(name="singles", bufs=1) as singles,
            tc.tile_pool(name="io", bufs=4) as io,
            tc.tile_pool(name="tpsum", bufs=2, space="PSUM") as tpsum,
            tc.tile_pool(name="spsum", bufs=2, space="PSUM") as spsum,
        ):
            wsb = singles.tile([128, NPAIRS * 128], mm_dt)
            # Weight table rides the Scalar HWDGE ring, in parallel with the
            # e1/e2 stream on the Sync ring; pair 0's matmul only waits on
            # the first 32KB chunk.
            nc.scalar.dma_start(wsb[:, :128], wst_d[:, :128])
            nc.scalar.dma_start(wsb[:, 128:], wst_d[:, 128:])
            # Half-indicator "ones" weights, built on-device (saves a DMA).
            onesb = singles.tile([128, 2], vv_dt)
            nc.vector.memset(onesb, 0.0)
            nc.vector.memset(onesb[0:64, 0:1], 1.0)
            nc.vector.memset(onesb[64:128, 1:2], 1.0)

            v = singles.tile([128, W], vv_dt)
            scores = singles.tile([2, W], mybir.dt.float32)

            OUT_GROUP = 4  # stream scores out every OUT_GROUP reduce chunks
            SP_GROUP = 2  # reduce chunks per PSUM score tile / ACT copy
            next_chunk = [0]
            last_sent = [0]

            def emit_reduce_upto(pair_done, flush=False):
                # Emit reduce chunk groups whose straddling pairs' multiplies
                # were emitted >= REDUCE_LAG pairs ago, so the in-order PE
                # queue never stalls waiting on a just-issued vector multiply.
                while next_chunk[0] < n_chunks:
                    j0 = next_chunk[0]
                    jn = min(j0 + SP_GROUP, n_chunks)
                    last_pair = (jn * 512 - 1) // C
                    if pair_done < NPAIRS - 1 and last_pair > pair_done:
                        break
                    sp = spsum.tile([2, SP_GROUP * 512], mybir.dt.float32, tag="sp")
                    for j in range(j0, jn):
                        nc.tensor.matmul(
                            sp[:, (j - j0) * 512 : (j - j0 + 1) * 512],
                            onesb,
                            v[:, j * 512 : (j + 1) * 512],
                            start=True,
                            stop=True,
                        )
                    nc.scalar.copy(
                        scores[:, j0 * 512 : jn * 512], sp[:, : (jn - j0) * 512]
                    )
                    if jn % OUT_GROUP == 0 or jn == n_chunks:
                        # Mid-stream: GpSimd SWDGE, so these copy-dependent
                        # stores can't head-of-line-block the input loads on
                        # the Sync ring. After all loads are emitted (flush),
                        # the faster Sync HWDGE ring is free again.
                        g0 = last_sent[0] * 512
                        eng = nc.sync if flush else nc.gpsimd
                        eng.dma_start(
                            out_d[:, g0 : jn * 512], scores[:, g0 : jn * 512]
                        )
                        last_sent[0] = jn
                    next_chunk[0] = jn

            # DMA in groups of pairs per transfer. Small groups up front so
            # the DVE's multiply pipeline starts as soon as data trickles in
            # (per-pair HBM arrival ~= per-pair DVE cost); bigger groups
            # later amortize descriptor overhead.
            GROUPS = [1, 3, 4, 4, 4]
            REDUCE_LAG = 2
            MAXG = max(GROUPS)
            g = 0
            for gi, npair in enumerate(GROUPS):
                gc = npair * C
                e1g = io.tile([128, MAXG * C], mm_dt, tag="e1g")
                e2g = io.tile([128, MAXG * C], ee_dt, tag="e2g")
                # Both input streams ride the Sync HWDGE ring: the Scalar
                # ring's descriptor-gen would head-of-line-block behind the
                # data-dependent score copies on the ACT sequencer.
                nc.sync.dma_start(e1g[:, :gc], e1t_d[:, g * C : g * C + gc])
                nc.sync.dma_start(e2g[:, :gc], e2t_d[:, g * C : g * C + gc])

                for lp in range(npair):
                    p = g + lp
                    w_p = wsb[:, p * 128 : (p + 1) * 128]

                    # PSUM tiles cover up to 1024 columns (2 banks); matmuls
                    # are issued per <=512-column sub-chunk so each stays in
                    # one bank.
                    for base in range(0, C, 1024):
                        sc = min(1024, C - base)
                        tp = tpsum.tile([128, sc], mybir.dt.float32, tag="tp")
                        for off in range(0, sc, 512):
                            sz = min(512, sc - off)
                            nc.tensor.matmul(
                                tp[:, off : off + sz],
                                w_p,
                                e1g[:, lp * C + base + off : lp * C + base + off + sz],
                                start=True,
                                stop=True,
                            )
                        nc.vector.tensor_mul(
                            out=v[:, p * C + base : p * C + base + sc],
                            in0=tp,
                            in1=e2g[:, lp * C + base : lp * C + base + sc],
                        )
                    emit_reduce_upto(p - REDUCE_LAG)
                g += npair

            emit_reduce_upto(NPAIRS - 1, flush=True)

    nc.compile()
    return nc, W, n_chunks


def _prep_core(e1, e2, my_idx, my_r, C):
    """Lay out one core's (already relation-sorted) samples into the
    stacked/padded [128, W] transposed arrays. Returns the (half, col)
    address of every sample in my_idx."""
    W = NPAIRS * C
    n = len(my_idx)
    seg_starts = np.searchsorted(my_r, np.arange(NUM_REL), side="left")
    pos_in_seg = np.arange(n, dtype=np.int64) - seg_starts[my_r]
    half = (my_r // NPAIRS).astype(np.int64)  # 0: relations 0-15, 1: 16-31
    col = (my_r % NPAIRS).astype(np.int64) * C + pos_in_seg

    e1t = np.zeros((128, W), dtype=np.float32)
    e2t = np.zeros((128, W), dtype=np.float32)
    for h in (0, 1):
        m = half == h
        rows = slice(64 * h, 64 * h + 64)
        e1t[rows, col[m]] = e1[my_idx[m]].T
        e2t[rows, col[m]] = e2[my_idx[m]].T
    return e1t, e2t, half, col


def kernel(embeds1, embeds2, rels, rel_embeds):
    global LAST_RESULTS
    from concourse.bass_utils import run_bass_kernel_spmd

    e1 = np.ascontiguousarray(np.asarray(embeds1, dtype=np.float32))
    e2 = np.ascontiguousarray(np.asarray(embeds2, dtype=np.float32))
    r = np.asarray(rels).astype(np.int64)
    rel_w = np.asarray(rel_embeds, dtype=np.float32)

    assert e1.shape == (B, D) and e2.shape == (B, D) and r.shape == (B,)
    assert rel_w.shape == (NUM_REL, D * D)

    # Globally sort by relation, then deal round-robin to cores: per-core
    # per-relation counts end up within +-1 of total_r/8, minimizing the
    # padded capacity C (multiple of 64).
    order_g = np.argsort(r, kind="stable")
    sorted_r = r[order_g]
    totals = np.bincount(r, minlength=NUM_REL)
    # C need only be a multiple of 32 (W = 16*C stays a multiple of 512).
    C = int(-(-max(1, int(-(-totals.max() // NCORES))) // 32) * 32)

    nc, W, n_chunks = _get_compiled(C)

    s1_np = _np_dt(STAGE1_DT)
    v_np = _np_dt(V_DT)
    e2_np = _np_dt(E2_DT)

    # Block-diagonal stationary weights: wst[k, p*128 + m]
    wst = np.zeros((128, NPAIRS * 128), dtype=np.float32)
    for p in range(NPAIRS):
        wst[:64, p * 128 : p * 128 + 64] = rel_w[p].reshape(D, D)
        wst[64:, p * 128 + 64 : p * 128 + 128] = rel_w[p + NPAIRS].reshape(D, D)
    wst = wst.astype(s1_np)

    in_maps = []
    addr = []
    for c in range(NCORES):
        my_idx = order_g[c::NCORES]  # this core's samples, relation-sorted
        my_r = sorted_r[c::NCORES]
        e1t, e2t, half, col = _prep_core(e1, e2, my_idx, my_r, C)
        in_maps.append(
            {
                "e1t": e1t.astype(s1_np),
                "e2t": e2t.astype(e2_np),
                "wst": wst,
            }
        )
        addr.append((my_idx, half, col))

    res = run_bass_kernel_spmd(nc, in_maps, core_ids=list(range(NCORES)), trace=TRACE)
    LAST_RESULTS = res

    out = np.empty(B, dtype=np.float32)
    for c in range(NCORES):
        packed = np.asarray(res.results[c]["out"])  # [2, W]
        my_idx, half, col = addr[c]
        out[my_idx] = packed[half, col]
    return out


def _get_compiled(C):
    key = (C, STAGE1_DT, V_DT, E2_DT)
    if key not in _compiled_cache:
        _compiled_cache[key] = _build_nc(C, STAGE1_DT, V_DT, E2_DT)
    return _compiled_cache[key]


# revision 61
# speedup vs baseline: 1.1018x; 1.1018x over previous
"""Trainium2 Bass kernel for nn_BilinearDecoder (B=131072, D=64, 32 relations).

score_b = e1_b^T R[rels_b] e2_b

Strategy
--------
Data-parallel over 8 NeuronCores (16384 samples/core). On the host, each
core's samples are counting-sorted by relation id and padded so every
relation owns a fixed-capacity column range. Relations are paired
(r, r+16): relation r lives on SBUF partitions 0..63 ("top half"),
relation r+16 on partitions 64..127 ("bottom half"), so every engine runs
at the full 128-partition width.

Per relation pair p the device computes, with a single 128x128
block-diagonal stationary weight W_p = diag(R_p, R_{p+16}):

    t[:, b]  = W_p^T e1t[:, b]          (TensorE, bf16 in / fp32 PSUM)
    v[:, b]  = t[:, b] * e2t[:, b]      (VectorE, PSUM x SBUF -> SBUF)
    score_b  = ones_half^T v[:, b]      (TensorE reduce with [128,2] ones)

Everything data-dependent (the gather by relation id) happens in host
numpy as part of sharding; the device only does dense work. The reference
value of each padded column is 0 and is discarded on the host, which also
inverts the permutation to restore the original sample order.
"""

import numpy as np

B = 131072
D = 64
NUM_REL = 32
NCORES = 8
NPAIRS = NUM_REL // 2  # 16 relation pairs -> halves of the 128 partitions

# Module-level switches test.py may flip; the graded entry point is kernel().
TRACE = False
LAST_RESULTS = None

# Dtype configuration: stage-1 matmul operands, intermediate v (reduce
# matmul operands), and the e2 vector-multiply operand. All-bf16 halves
# HBM traffic and runs every matmul at 1 cycle/row; measured end-to-end
# scaled error vs the fp32 reference is ~3e-3 (fp32 PSUM accumulation).
STAGE1_DT = "bfloat16"
V_DT = "bfloat16"
E2_DT = "bfloat16"

_compiled_cache = {}


def _np_dt(name):
    if name == "bfloat16":
        import ml_dtypes

        return ml_dtypes.bfloat16
    return np.float32  # float32 and float32r share the numpy layout


def _build_nc(C, stage1_dt="bfloat16", v_dt="bfloat16", e2_dt="bfloat16"):
    """Build + compile the SPMD program for per-relation capacity C.

    C is a multiple of 32. Column grid per core: W = NPAIRS * C, where
    pair p owns columns [p*C, (p+1)*C) and carries relations p (top 64
    partitions) and p+16 (bottom 64 partitions).

    stage1_dt: dtype of e1 / weights feeding the block-diagonal matmul.
    v_dt: dtype of the intermediate v (= t * e2) feeding the ones-reduce
          matmul. bf16 runs the reduce at 1 cycle/row (fp32r runs at 4).
    e2_dt: dtype of the streamed e2 operand of the vector multiply.
    """
    import concourse.bass as bass  # noqa: F401  (registers engine classes)
    import concourse.mybir as mybir
    import concourse.tile as tile
    from concourse import bacc

    W = NPAIRS * C
    n_chunks = W // 512  # W = 16*C, C % 32 == 0 -> W % 512 == 0
    # SBUF budget guard: v is [128, W] bf16 plus double-buffered input
    # groups; far beyond any plausible capacity for uniform relation ids.
    assert W <= 24576, f"relation distribution too skewed for this layout (C={C})"
    mm_dt = getattr(mybir.dt, stage1_dt)
    vv_dt = getattr(mybir.dt, v_dt)
    ee_dt = getattr(mybir.dt, e2_dt)

    nc = bacc.Bacc("TRN2", debug=False, num_devices=NCORES)

    # The BIR verifier requires fp32r matmul operands' producers to emit
    # fp32r-rounded values, so PE-feeding tensors are declared in the
    # matmul dtype end-to-end (fp32r has the same 4-byte numpy layout).
    e1t_d = nc.dram_tensor("e1t", [128, W], mm_dt, kind="ExternalInput").ap()
    e2t_d = nc.dram_tensor("e2t", [128, W], ee_dt, kind="ExternalInput").ap()
    wst_d = nc.dram_tensor(
        "wst", [128, NPAIRS * 128], mm_dt, kind="ExternalInput"
    ).ap()

    # Score layout: row 0 = top-half relations, row 1 = bottom-half.
    # (Compute-engine APs must start at a partition multiple of 32, so the
    # scores stay on partitions 0-1.)
    out_d = nc.dram_tensor("out", [2, W], mybir.dt.float32, kind="ExternalOutput").ap()

    with tile.TileContext(nc) as tc:
        with (
            tc.tile_pool(name="singles", bufs=1) as singles,
            tc.tile_pool(name="io", bufs=5) as io,
            tc.tile_pool(name="tpsum", bufs=2, space="PSUM") as tpsum,
            tc.tile_pool(name="spsum", bufs=2, space="PSUM") as spsum,
        ):
            wsb = singles.tile([128, NPAIRS * 128], mm_dt)
            # Weight table rides the Scalar HWDGE ring, in parallel with the
            # e1/e2 stream on the Sync ring; pair 0's matmul only waits on
            # the first 32KB chunk.
            nc.scalar.dma_start(wsb[:, :128], wst_d[:, :128])
            nc.scalar.dma_start(wsb[:, 128:], wst_d[:, 128:])
            # Half-indicator "ones" weights, built on-device (saves a DMA).
            onesb = singles.tile([128, 2], vv_dt)
            nc.vector.memset(onesb, 0.0)
            nc.vector.memset(onesb[0:64, 0:1], 1.0)
            nc.vector.memset(onesb[64:128, 1:2], 1.0)

            v = singles.tile([128, W], vv_dt)
            scores = singles.tile([2, W], mybir.dt.float32)

            OUT_GROUP = 4  # stream scores out every OUT_GROUP reduce chunks
            SP_GROUP = 2  # reduce chunks per PSUM score tile / ACT copy
            next_chunk = [0]
            last_sent = [0]

            def emit_reduce_upto(pair_done, flush=False):
                # Emit reduce chunk groups whose straddling pairs' multiplies
                # were emitted >= REDUCE_LAG pairs ago, so the in-order PE
                # queue never stalls waiting on a just-issued vector multiply.
                while next_chunk[0] < n_chunks:
                    j0 = next_chunk[0]
                    jn = min(j0 + SP_GROUP, n_chunks)
                    last_pair = (jn * 512 - 1) // C
                    if pair_done < NPAIRS - 1 and last_pair > pair_done:
                        break
                    sp = spsum.tile([2, SP_GROUP * 512], mybir.dt.float32, tag="sp")
                    for j in range(j0, jn):
                        nc.tensor.matmul(
                            sp[:, (j - j0) * 512 : (j - j0 + 1) * 512],
                            onesb,
                            v[:, j * 512 : (j + 1) * 512],
                            start=True,
                            stop=True,
                        )
                    nc.scalar.copy(
                        scores[:, j0 * 512 : jn * 512], sp[:, : (jn - j0) * 512]
                    )
                    if jn % OUT_GROUP == 0 or jn == n_chunks:
                        # Mid-stream: GpSimd SWDGE, so these copy-dependent
                        # stores can't head-of-line-block the input loads on
                        # the Sync ring. After all loads are emitted (flush),
                        # the faster Sync HWDGE ring is free again.
                        g0 = last_sent[0] * 512
                        eng = nc.sync if flush else nc.gpsimd
                        eng.dma_start(
                            out_d[:, g0 : jn * 512], scores[:, g0 : jn * 512]
                        )
                        last_sent[0] = jn
                    next_chunk[0] = jn

            # DMA in groups of pairs per transfer. Small groups up front so
            # the DVE's multiply pipeline starts as soon as data trickles in
            # (per-pair HBM arrival ~= per-pair DVE cost); bigger groups
            # later amortize descriptor overhead.
            GROUPS = [1, 3, 4, 4, 4]
            REDUCE_LAG = 2
            MAXG = max(GROUPS)
            g = 0
            for gi, npair in enumerate(GROUPS):
                gc = npair * C
                e1g = io.tile([128, MAXG * C], mm_dt, tag="e1g")
                e2g = io.tile([128, MAXG * C], ee_dt, tag="e2g")
                # Both input streams ride the Sync HWDGE ring: the Scalar
                # ring's descriptor-gen would head-of-line-block behind the
                # data-dependent score copies on the ACT sequencer.
                nc.sync.dma_start(e1g[:, :gc], e1t_d[:, g * C : g * C + gc])
                nc.sync.dma_start(e2g[:, :gc], e2t_d[:, g * C : g * C + gc])

                for lp in range(npair):
                    p = g + lp
                    w_p = wsb[:, p * 128 : (p + 1) * 128]

                    # PSUM tiles cover up to 1024 columns (2 banks); matmuls
                    # are issued per <=512-column sub-chunk so each stays in
                    # one bank.
                    for base in range(0, C, 1024):
                        sc = min(1024, C - base)
                        tp = tpsum.tile([128, sc], mybir.dt.float32, tag="tp")
                        for off in range(0, sc, 512):
                            sz = min(512, sc - off)
                            nc.tensor.matmul(
                                tp[:, off : off + sz],
                                w_p,
                                e1g[:, lp * C + base + off : lp * C + base + off + sz],
                                start=True,
                                stop=True,
                            )
                        nc.vector.tensor_mul(
                            out=v[:, p * C + base : p * C + base + sc],
                            in0=tp,
                            in1=e2g[:, lp * C + base : lp * C + base + sc],
                        )
                    emit_reduce_upto(p - REDUCE_LAG)
                g += npair

            emit_reduce_upto(NPAIRS - 1, flush=True)

    nc.compile()
    return nc, W, n_chunks


def _prep_core(e1, e2, my_idx, my_r, C):
    """Lay out one core's (already relation-sorted) samples into the
    stacked/padded [128, W] transposed arrays. Returns the (half, col)
    address of every sample in my_idx."""
    W = NPAIRS * C
    n = len(my_idx)
    seg_starts = np.searchsorted(my_r, np.arange(NUM_REL), side="left")
    pos_in_seg = np.arange(n, dtype=np.int64) - seg_starts[my_r]
    half = (my_r // NPAIRS).astype(np.int64)  # 0: relations 0-15, 1: 16-31
    col = (my_r % NPAIRS).astype(np.int64) * C + pos_in_seg

    e1t = np.zeros((128, W), dtype=np.float32)
    e2t = np.zeros((128, W), dtype=np.float32)
    for h in (0, 1):
        m = half == h
        rows = slice(64 * h, 64 * h + 64)
        e1t[rows, col[m]] = e1[my_idx[m]].T
        e2t[rows, col[m]] = e2[my_idx[m]].T
    return e1t, e2t, half, col


def kernel(embeds1, embeds2, rels, rel_embeds):
    global LAST_RESULTS
    from concourse.bass_utils import run_bass_kernel_spmd

    e1 = np.ascontiguousarray(np.asarray(embeds1, dtype=np.float32))
    e2 = np.ascontiguousarray(np.asarray(embeds2, dtype=np.float32))
    r = np.asarray(rels).astype(np.int64)
    rel_w = np.asarray(rel_embeds, dtype=np.float32)

    assert e1.shape == (B, D) and e2.shape == (B, D) and r.shape == (B,)
    assert rel_w.shape == (NUM_REL, D * D)

    # Globally sort by relation, then deal round-robin to cores: per-core
    # per-relation counts end up within +-1 of total_r/8, minimizing the
    # padded capacity C (multiple of 64).
    order_g = np.argsort(r, kind="stable")
    sorted_r = r[order_g]
    totals = np.bincount(r, minlength=NUM_REL)
    # C need only be a multiple of 32 (W = 16*C stays a multiple of 512).
    C = int(-(-max(1, int(-(-totals.max() // NCORES))) // 32) * 32)

    nc, W, n_chunks = _get_compiled(C)

    s1_np = _np_dt(STAGE1_DT)
    v_np = _np_dt(V_DT)
    e2_np = _np_dt(E2_DT)

    # Block-diagonal stationary weights: wst[k, p*128 + m]
    wst = np.zeros((128, NPAIRS * 128), dtype=np.float32)
    for p in range(NPAIRS):
        wst[:64, p * 128 : p * 128 + 64] = rel_w[p].reshape(D, D)
        wst[64:, p * 128 + 64 : p * 128 + 128] = rel_w[p + NPAIRS].reshape(D, D)
    wst = wst.astype(s1_np)

    in_maps = []
    addr = []
    for c in range(NCORES):
        my_idx = order_g[c::NCORES]  # this core's samples, relation-sorted
        my_r = sorted_r[c::NCORES]
        e1t, e2t, half, col = _prep_core(e1, e2, my_idx, my_r, C)
        in_maps.append(
            {
                "e1t": e1t.astype(s1_np),
                "e2t": e2t.astype(e2_np),
                "wst": wst,
            }
        )
        addr.append((my_idx, half, col))

    res = run_bass_kernel_spmd(nc, in_maps, core_ids=list(range(NCORES)), trace=TRACE)
    LAST_RESULTS = res

    out = np.empty(B, dtype=np.float32)
    for c in range(NCORES):
        packed = np.asarray(res.results[c]["out"])  # [2, W]
        my_idx, half, col = addr[c]
        out[my_idx] = packed[half, col]
    return out


def _get_compiled(C):
    key = (C, STAGE1_DT, V_DT, E2_DT)
    if key not in _compiled_cache:
        _compiled_cache[key] = _build_nc(C, STAGE1_DT, V_DT, E2_DT)
    return _compiled_cache[key]
